# revision 59
# baseline (speedup 1.0000x reference)
"""Trainium2 Bass kernel for nn_AddingLRU (LRU scan + masked last-step readout).

Math: the reference's output only depends on the LRU state at t = lengths[b]-1
per batch element:
    out[b] = Re(WC . state_b) + WD . x[b, l_b-1] + bh,
    state_b[n] = sum_{t < l_b} lam[n]^(l_b-1-t) * (Bmat[n,:] . x[b,t,:])
with WC = Wh@C, WD = Wh@D, Bmat = (B_re + i B_im)*gamma.  After a host-side
time-reversal gather (xrev_b[k] = x[b, l_b-1-k]), the state is
    state_b[n] = Bmat[n,:] . sum_k lam[n]^k xrev_b[k,:]
and with k = 128c + j the inner sum factors into a [128,128]x[128,128] matmul
against the power table QT[j,n] = lam[n]^j followed by a weighted reduction
with W[n,c] = lam[n]^(128c).  Both tables are built on-device from
nu_log/theta_log via exp/sin with explicit range reduction to [-pi, pi].

Sharding: data-parallel over batch, 2 batch elements per NeuronCore x 8 cores.
"""
import os
import sys

import numpy as np

for _p in ("/opt/trn_rl_repo", "/root/.axon_site/_ro/trn_rl_repo"):
    if os.path.isdir(_p) and _p not in sys.path:
        sys.path.append(_p)

from concourse import bacc, tile, mybir  # noqa: E402
from concourse.tile import add_dep_helper  # noqa: E402
from concourse.bass_utils import run_bass_kernel_spmd  # noqa: E402

F32 = mybir.dt.float32
TWO_PI = np.float32(6.283185307179586)
INV2PI = np.float32(1.0 / 6.283185307179586)
MAGIC = np.float32(12582912.0)  # 1.5 * 2**23: (x+M)-M == round-to-nearest(x)
SAFE_PI = np.float32(3.141590)  # just under fp32 pi; Sin valid range is [-pi, pi]
HALF_PI = np.float32(1.5707963705062866)

B_, T_, IN_, H_, N_ = 16, 4096, 2, 256, 256
NCORES = 8
NB = B_ // NCORES  # batches per core = 2

LAST_RESULT = None  # BassKernelResults of the most recent run (for test harness)
_NC_CACHE = None  # compiled Bass graph, reused across kernel() calls


def _emit_rsin(nc, sb, out, src, bias, tag, eng=None, eng_head=None):
    """out = sin(src + bias) elementwise, with range reduction to [-pi, pi].

    q = rne((src + bias)/2pi);  r = src - 2pi*q;  out = Sin(clamp(r) + bias).
    bias must be 0.0 or pi/2 (float).  src/out: SBUF APs of equal shape.
    eng_head runs the independent q/k prefix (e.g. on Pool for overlap).
    """
    shape = list(src.shape)
    if eng is None:
        eng = nc.vector
    if eng_head is None:
        eng_head = eng
    q = sb.tile(shape, F32, tag=f"{tag}_q", name=f"{tag}_q")
    if bias == 0.0:
        # q = src*inv2pi + MAGIC  (fused)
        eng_head.tensor_scalar(q[:, :], src, float(INV2PI), float(MAGIC),
                               mybir.AluOpType.mult, mybir.AluOpType.add)
    else:
        boff = np.float32(bias * float(INV2PI))
        eng_head.tensor_scalar(q[:, :], src, float(INV2PI), float(boff),
                               mybir.AluOpType.mult, mybir.AluOpType.add)
        eng_head.tensor_scalar_add(q[:, :], q[:, :], float(MAGIC))
    k = sb.tile(shape, F32, tag=f"{tag}_k", name=f"{tag}_k")
    head_last = eng_head.tensor_scalar_sub(k[:, :], q[:, :], float(MAGIC))
    rr = sb.tile(shape, F32, tag=f"{tag}_r", name=f"{tag}_r")
    # rr = k*(-2pi) + src
    eng.scalar_tensor_tensor(rr[:, :], k[:, :], -float(TWO_PI), src,
                                   mybir.AluOpType.mult, mybir.AluOpType.add)
    # clamp so (rr + bias) stays inside [-pi, pi] for the Sin table
    hi = float(SAFE_PI) - float(bias)
    lo = -float(SAFE_PI) - float(bias)
    last = eng.tensor_scalar(rr[:, :], rr[:, :], hi, lo,
                             mybir.AluOpType.min, mybir.AluOpType.max)
    if bias != 0.0:
        last = eng.tensor_scalar_add(rr[:, :], rr[:, :], float(bias))
    return (nc.scalar.activation(out, rr[:, :],
                                 mybir.ActivationFunctionType.Sin),
            last, head_last, rr)


def build_graph():
    nc = bacc.Bacc(None, target_bir_lowering=False, debug=False)

    # ---- DRAM parameters (host packs small tensors to minimize DMA count)
    # pkrow [1, 512]: nu_log row (0:256) | theta_log row (256:512)
    # pk [128, 21]: nulog_col(0:2) thetalog_col(2:4) bre(4:8) bim(8:12)
    #              d(12:16) wh(16:18) bh(p0, 18) xlast(p0:2, 19:21)
    # c2 [128, 1024]: cre (0:512) | cim (512:1024)
    pkrow_e = nc.declare_dram_parameter("pkrow", [1, 512], F32, isOutput=False)
    pk_e = nc.declare_dram_parameter("pk", [128, 21], F32, isOutput=False)
    xr_e = nc.declare_dram_parameter("xr", [128, 128], F32, isOutput=False)
    c2_e = nc.declare_dram_parameter("c2", [128, 1024], F32, isOutput=False)
    out_e = nc.declare_dram_parameter("out", [1, 2], F32, isOutput=True)

    MULT = mybir.AluOpType.mult
    ADD = mybir.AluOpType.add
    Exp = mybir.ActivationFunctionType.Exp
    Ln = mybir.ActivationFunctionType.Ln

    with tile.TileContext(nc) as tc:
        with tc.tile_pool(name="sb", bufs=1) as sb, \
             tc.tile_pool(name="ps", bufs=1, space="PSUM") as ps:
            def sbt(shape, tag):
                return sb.tile(shape, F32, tag=tag, name=tag)

            # ---- DMA inputs (critical-path first) ----
            # broadcast nu_log/theta_log rows across partitions straight from
            # DRAM (stride-0 free-dim read) -- no ACT dependency, issues at t=0
            bc_thlog = sbt([128, 256], "bc_thlog")
            nc.sync.dma_start(out=bc_thlog[:, :],
                              in_=pkrow_e[0:1, 256:512].unsqueeze(1)
                              .broadcast_to((1, 128, 256)))
            bc_nulog = sbt([128, 256], "bc_nulog")
            nc.sync.dma_start(out=bc_nulog[:, :],
                              in_=pkrow_e[0:1, 0:256].unsqueeze(1)
                              .broadcast_to((1, 128, 256)))
            pk = sbt([128, 21], "pk")
            nc.sync.dma_start(out=pk[:, :], in_=pk_e[:, :])
            xr = sbt([128, 128], "xr")
            c2 = sbt([128, 1024], "c2")
            nulc = pk[:, 0:2]
            thlc = pk[:, 2:4]
            bre_ap = pk[:, 4:8]
            bim_ap = pk[:, 8:12]
            dsb = pk[:, 12:16]
            wh = pk[:, 16:18]
            bh = pk[0:1, 18:19]
            xlast = pk[0:2, 19:21]
            cre = c2[:, 0:512]
            cim = c2[:, 512:1024]

            # ---- constants ----
            t_col = sbt([128, 1], "t_col")
            nc.gpsimd.iota(t_col[:, :], [[1, 1]], channel_multiplier=1,
                           allow_small_or_imprecise_dtypes=True)
            iota_w = sbt([128, 32], "iota_w")  # value = 128*c, same per partition
            nc.gpsimd.iota(iota_w[:, :], [[128, 32]], channel_multiplier=0,
                           allow_small_or_imprecise_dtypes=True)
            ones_col = sbt([128, 1], "ones_col")
            nc.gpsimd.memset(ones_col[:, :], 1.0)

            # ---- exp-family ACT batch ----
            th_bc = sbt([128, 256], "th_bc")  # theta broadcast [t, n]
            e_i2 = nc.scalar.activation(th_bc[:, :], bc_thlog[:, :], Exp)
            nu_bc = sbt([128, 256], "nu_bc")
            e_i1 = nc.scalar.activation(nu_bc[:, :], bc_nulog[:, :], Exp)
            a_col = sbt([128, 2], "a_col")  # nu = exp(nu_log), column halves
            e_i3 = nc.scalar.activation(a_col[:, :], nulc, Exp)
            th_col = sbt([128, 2], "th_col")
            e_i4 = nc.scalar.activation(th_col[:, :], thlc, Exp)
            # th_bc/nu_bc gate the DVE phase chain; keep them ahead of the
            # pk-gated column exps in the in-order ACT stream
            for late in (e_i3, e_i4):
                for early in (e_i2, e_i1):
                    add_dep_helper(late.ins, early.ins, sync=False,
                                   reason="broadcast exps first on ACT")
            negnu = sbt([128, 2], "negnu")
            negnu_i = nc.gpsimd.tensor_scalar_mul(negnu[:, :], a_col[:, :], -1.0)
            e2 = sbt([128, 2], "e2")  # exp(-2 nu) = |lam|^2
            e_i5 = nc.scalar.activation(e2[:, :], a_col[:, :], Exp, scale=-2.0)
            # gamma = sqrt(u), u = 1-e2, via bit-trick rsqrt + 2 Newton steps
            # (avoids the Ln/Sqrt ACT tables; only exp+sin sets get loaded)
            SHR = mybir.AluOpType.arith_shift_right
            u_g = sbt([128, 2], "u_g")
            u_g_i = nc.gpsimd.tensor_scalar(u_g[:, :], e2[:, :], -1.0, 1.0, MULT, ADD)
            uh_g = sbt([128, 2], "uh_g")
            nc.gpsimd.tensor_scalar_mul(uh_g[:, :], u_g[:, :], 0.5)
            yi_g = sb.tile([128, 2], mybir.dt.int32, tag="yi_g", name="yi_g")
            yi_i1 = nc.vector.tensor_scalar(yi_g[:, :],
                                            u_g[:, :].bitcast(mybir.dt.int32),
                                            1, None, SHR)
            # y0 bits = magic - (bits >> 1)
            yi_i2 = nc.vector.tensor_scalar(yi_g[:, :], yi_g[:, :], -1,
                                            0x5f3759df, MULT, ADD)
            y_g = sbt([128, 2], "y_g")
            nc.gpsimd.tensor_copy(y_g[:, :], yi_g[:, :].bitcast(F32))
            t_g = sbt([128, 2], "t_g")
            for _ in range(2):
                nc.gpsimd.tensor_mul(t_g[:, :], y_g[:, :], y_g[:, :])
                nc.gpsimd.tensor_mul(t_g[:, :], t_g[:, :], uh_g[:, :])
                nc.gpsimd.tensor_scalar(t_g[:, :], t_g[:, :], -1.0, 1.5, MULT, ADD)
                nc.gpsimd.tensor_mul(y_g[:, :], y_g[:, :], t_g[:, :])
            gamma = sbt([128, 2], "gamma")
            nc.gpsimd.tensor_mul(gamma[:, :], u_g[:, :], y_g[:, :])

            # bulk DMAs (needed much later) -- on the Pool DMA queue so their
            # completion semaphore doesn't gate the critical ACT ops
            nc.sync.dma_start(out=xr[:, :], in_=xr_e[:, :])
            nc.sync.dma_start(out=c2[:, :], in_=c2_e[:, :])
            bc_th = th_bc[:, :]
            bc_nu = nu_bc[:, :]

            # ---- phases & magnitudes, batched in one [128, 320] tile ----
            # cols 0:256   -> QT:  phase j*theta[n],  mag exp(-j*nu[n])
            # cols 256+32h -> W_h: phase 128c*theta_h, mag exp(-128c*nu_h)
            Cp = mybir.ActivationFunctionType.Copy
            P_all = sbt([128, 320], "P_all")
            nc.vector.tensor_scalar(P_all[:, 0:256], bc_th, t_col[:, 0:1],
                                    None, MULT)
            for h in (0, 1):
                nc.vector.tensor_scalar(P_all[:, 256 + 32 * h:288 + 32 * h],
                                        iota_w[:, :], th_col[:, h:h + 1],
                                        None, MULT)
            negt = sbt([128, 1], "negt")
            nc.gpsimd.tensor_scalar_mul(negt[:, :], t_col[:, :], -1.0)
            E_all = sbt([128, 320], "E_all")
            e_i8 = nc.scalar.activation(E_all[:, 0:256], bc_nu, Exp,
                                        scale=negt[:, 0:1])
            exp_insts = [e_i1, e_i2, e_i3, e_i4, e_i5, e_i8]
            for h in (0, 1):
                exp_insts.append(
                    nc.scalar.activation(E_all[:, 256 + 32 * h:288 + 32 * h],
                                         iota_w[:, :], Exp,
                                         scale=negnu[:, h:h + 1]))
            # PRD packs both tables: [0:320] = re = E*cos, [320:640] = im = E*sin
            # QTre = PRD[:, 0:256], QTim = PRD[:, 320:576]
            # Wre_h = PRD[:, 256+32h], Wim_h = PRD[:, 576+32h]  (im = re + 320)
            PRD = sbt([128, 640], "PRD")
            sinA = sbt([128, 320], "sinA")
            sin_i1, _, _, rr_sin = _emit_rsin(nc, sb, sinA[:, :], P_all[:, :],
                                             0.0, "sa")
            # PRD-im lands between the two rsin chains in the DVE stream so it
            # overlaps cosA's ACT call and unblocks the imaginary matmuls early
            nc.vector.tensor_mul(PRD[:, 320:640], E_all[:, :], sinA[:, :])
            # cos(P) = cos(|r_sin|) = sin(pi/2 - |r_sin|): reuse the sin
            # chain's reduced argument (2 DVE ops instead of a second chain)
            cosA = sbt([128, 320], "cosA")
            cabs = sb.tile([128, 320], mybir.dt.int32, tag="cabs", name="cabs")
            nc.vector.tensor_scalar(cabs[:, :],
                                    rr_sin[:, :].bitcast(mybir.dt.int32),
                                    0x7fffffff, None,
                                    mybir.AluOpType.bitwise_and)
            carg = sbt([128, 320], "carg")
            cos_last = nc.vector.tensor_scalar(carg[:, :],
                                               cabs[:, :].bitcast(F32),
                                               -1.0, float(HALF_PI), MULT, ADD)
            sin_i2 = nc.scalar.activation(cosA[:, :], carg[:, :],
                                          mybir.ActivationFunctionType.Sin)
            # -Wim for the T1 pt1 products (needs only the im half of PRD)
            PRDn = sbt([128, 64], "PRDn")
            nc.vector.tensor_scalar_mul(PRDn[:, :], PRD[:, 576:640], -1.0)
            nc.vector.tensor_mul(PRD[:, 0:320], E_all[:, :], cosA[:, :])
            for ei in exp_insts:
                add_dep_helper(sin_i1.ins, ei.ins, sync=False,
                               reason="batch exp table before sin table")
                add_dep_helper(sin_i2.ins, ei.ins, sync=False,
                               reason="batch exp table before sin table")
            # keep the off-critical-path int rsqrt ops behind the rsin chains
            add_dep_helper(yi_i1.ins, cos_last.ins, sync=False,
                           reason="defer rsqrt int ops on DVE")
            add_dep_helper(yi_i2.ins, cos_last.ins, sync=False,
                           reason="defer rsqrt int ops on DVE")

            # ---- main matmuls: hps[h][pt][n, (c, bi)] = QT^T @ xr ----
            # four PSUM tiles so each consumer waits only on its own matmul
            hps = [[None, None], [None, None]]
            hsb = [[None, None], [None, None]]  # SBUF bounce for Pool (no PSUM)
            hsb_last = None
            for pt in (1, 0):  # imaginary first: sinA finishes before cosA
                for h in (0, 1):
                    t = ps.tile([128, 128], F32, tag=f"hps{h}{pt}",
                                name=f"hps{h}{pt}")
                    hps[h][pt] = t
                    q = 320 * pt + 128 * h
                    nc.tensor.matmul(t[:, :], PRD[:, q:q + 128], xr[:, :])
                    hsb[h][pt] = sbt([128, 128], f"hsb{h}{pt}")
                    hsb_last = nc.scalar.activation(hsb[h][pt][:, :], t[:, :], Cp)
                    add_dep_helper(hsb_last.ins, sin_i2.ins, sync=False,
                                   reason="bounce copies after sins on ACT")


            # ---- combine: g = sum_c W * h (complex) ----
            # gre = sum_c (Wre*hre + (-Wim)*him): with -Wim folded into the
            # product weights, the (pt, c) reduce yields gre directly.
            # per-(h, pt) products into per-h tiles T1t[h] (f = pt*128+c*4+bi)
            T1t = [sbt([128, 256], "T1t0"), sbt([128, 256], "T1t1")]
            T2t = [sbt([128, 256], "T2t0"), sbt([128, 256], "T2t1")]
            wre_ap = lambda h: PRD[:, 256 + 32 * h:288 + 32 * h] \
                .rearrange("p c -> p c").unsqueeze(2).broadcast_to((128, 32, 4))
            wim_ap = lambda h: PRD[:, 576 + 32 * h:608 + 32 * h] \
                .unsqueeze(2).broadcast_to((128, 32, 4))
            wimn_ap = lambda h: PRDn[:, 32 * h:32 * h + 32] \
                .unsqueeze(2).broadcast_to((128, 32, 4))
            def t1v(h):
                return T1t[h][:, :].rearrange("p (pt c b) -> p pt c b",
                                              pt=2, c=32, b=4)

            def t2v(h):
                return T2t[h][:, :].rearrange("p (pt c b) -> p pt c b",
                                              pt=2, c=32, b=4)

            def hv(h, pt):
                return hps[h][pt][:, :].rearrange("p (c b) -> p c b", c=32, b=4)

            # T1 (DVE): pt0 = Wre*hre, pt1 = (-Wim)*him; im products first
            for h in (0, 1):
                nc.vector.tensor_tensor(t1v(h)[:, 1, :, :], wimn_ap(h),
                                        hv(h, 1), MULT)
            for h in (0, 1):
                nc.vector.tensor_tensor(t1v(h)[:, 0, :, :], wre_ap(h),
                                        hv(h, 0), MULT)
            # T2 (Pool): pt0 = Wre*him, pt1 = Wim*hre; him-based first
            def hvs(h, pt):
                return hsb[h][pt][:, :].rearrange("p (c b) -> p c b", c=32, b=4)

            t2_last = None
            for pt in (0, 1):
                for h in (0, 1):
                    t2_last = nc.gpsimd.tensor_tensor(
                        t2v(h)[:, pt, :, :],
                        wre_ap(h) if pt == 0 else wim_ap(h),
                        hvs(h, 1 - pt), MULT)
            # g2 = [gre | gim], each (h, bb, i); reduce over (pt, c)
            g2 = sbt([128, 16], "g2")
            for h in (0, 1):
                nc.vector.tensor_reduce(
                    g2[:, 4 * h:4 * h + 4],
                    T1t[h][:, :].rearrange("p (pt c b) -> p b pt c",
                                           pt=2, c=32, b=4),
                    mybir.AxisListType.XY, ADD)
            for h in (0, 1):
                nc.vector.tensor_reduce(
                    g2[:, 8 + 4 * h:12 + 4 * h],
                    T2t[h][:, :].rearrange("p (pt c b) -> p b pt c",
                                           pt=2, c=32, b=4),
                    mybir.AxisListType.XY, ADD)

            # ---- Bmat fold: state[n, (h, bb)] (prep is off the critical path)
            breg = sbt([128, 4], "breg")
            bimg = sbt([128, 4], "bimg")
            for h in (0, 1):
                bi1 = nc.scalar.activation(breg[:, 2 * h:2 * h + 2],
                                           bre_ap[:, 2 * h:2 * h + 2], Cp,
                                           scale=gamma[:, h:h + 1])
                bi2 = nc.scalar.activation(bimg[:, 2 * h:2 * h + 2],
                                           bim_ap[:, 2 * h:2 * h + 2], Cp,
                                           scale=gamma[:, h:h + 1])
                for late in (bi1, bi2):
                    add_dep_helper(late.ins, sin_i1.ins, sync=False,
                                   reason="keep crit-path ACTs ahead")
                    add_dep_helper(late.ins, hsb_last.ins, sync=False,
                                   reason="bounce copies first on ACT")
            bimgn = sbt([128, 4], "bimgn")
            nc.gpsimd.tensor_scalar_mul(bimgn[:, :], bimg[:, :], -1.0)
            # critical: P16a = g2 * [bregf | -bimgf], P16b = g2 * [bimgf | bregf]
            # bb-broadcast of breg/bimg(n) via stride-0 views (no materialization)
            bwv = lambda t: t[:, :].rearrange("p (h i) -> p h i", h=2, i=2)                 .unsqueeze(2).broadcast_to((128, 2, 2, 2))                 .rearrange("p h bb i -> p h bb i")
            g2v = g2[:, :].rearrange("p (pt h bb i) -> p pt h bb i",
                                     pt=2, h=2, bb=2, i=2)
            P16a = sbt([128, 16], "P16a")
            p16av = P16a[:, :].rearrange("p (pt h bb i) -> p pt h bb i",
                                         pt=2, h=2, bb=2, i=2)
            nc.vector.tensor_tensor(p16av[:, 0], g2v[:, 0], bwv(breg), MULT)
            nc.vector.tensor_tensor(p16av[:, 1], g2v[:, 1], bwv(bimgn), MULT)
            P16b = sbt([128, 16], "P16b")
            p16bv = P16b[:, :].rearrange("p (pt h bb i) -> p pt h bb i",
                                         pt=2, h=2, bb=2, i=2)
            nc.vector.tensor_tensor(p16bv[:, 0], g2v[:, 1], bwv(breg), MULT)
            nc.vector.tensor_tensor(p16bv[:, 1], g2v[:, 0], bwv(bimg), MULT)
            st2 = sbt([128, 8], "st2")
            nc.vector.tensor_reduce(
                st2[:, 0:4],
                P16a[:, :].rearrange("p (pt hb i) -> p hb pt i",
                                     pt=2, hb=4, i=2),
                mybir.AxisListType.XY, ADD)
            nc.vector.tensor_reduce(
                st2[:, 4:8],
                P16b[:, :].rearrange("p (pt hb i) -> p hb pt i",
                                     pt=2, hb=4, i=2),
                mybir.AxisListType.XY, ADD)

            # ---- WC = Wh @ C as columns [n-half], WD = Wh @ D ----
            wc_ps = ps.tile([128, 4], F32, tag="wc_ps", name="wc_ps")  # cols: pt*2 + h
            for pt, csb in ((0, cre), (1, cim)):
                for h in (0, 1):
                    col = 2 * pt + h
                    for hc in (0, 1):
                        o = 256 * hc + 128 * h
                        nc.tensor.matmul(wc_ps[:, col:col + 1],
                                         csb[:, o:o + 128], wh[:, hc:hc + 1],
                                         start=(hc == 0), stop=(hc == 1))
            # w_fin = [wcre | -wcim] bb-duplicated; f = pt*4 + h*2 + bb
            w_fin = sbt([128, 8], "w_fin")
            wf_v = w_fin[:, :].rearrange("p (pt h bb) -> p pt bb h",
                                         pt=2, h=2, bb=2)
            for bb in (0, 1):
                wfi1 = nc.scalar.activation(wf_v[:, 0, bb, :], wc_ps[:, 0:2], Cp)
                wfi2 = nc.scalar.activation(wf_v[:, 1, bb, :], wc_ps[:, 2:4], Cp,
                                            scale=-1.0)
                add_dep_helper(wfi1.ins, hsb_last.ins, sync=False,
                               reason="keep crit-path ACTs ahead")
                add_dep_helper(wfi2.ins, hsb_last.ins, sync=False,
                               reason="keep crit-path ACTs ahead")

            wd_ps = ps.tile([2, 1], F32, tag="wd_ps", name="wd_ps")
            for hc in (0, 1):
                nc.tensor.matmul(wd_ps[:, :], dsb[:, 2 * hc:2 * hc + 2],
                                 wh[:, hc:hc + 1],
                                 start=(hc == 0), stop=(hc == 1))
            wd_sb = sbt([2, 1], "wd_sb")
            wdi = nc.scalar.activation(wd_sb[:, :], wd_ps[:, :], Cp)
            add_dep_helper(wdi.ins, hsb_last.ins, sync=False,
                           reason="keep crit-path ACTs ahead")

            # ---- final: P8 = st2*w_fin; one PE reduce + one DVE reduce ----
            P8 = sbt([128, 8], "P8")
            nc.vector.tensor_mul(P8[:, :], st2[:, :], w_fin[:, :])
            ones2 = sbt([1, 2], "ones2")
            nc.gpsimd.memset(ones2[:, :], 1.0)
            fin_ps = ps.tile([1, 12], F32, tag="fin_ps", name="fin_ps")
            nc.tensor.matmul(fin_ps[:, 0:8], ones_col[:, :], P8[:, :])
            nc.tensor.matmul(fin_ps[:, 8:10], wd_sb[:, :], xlast)
            nc.tensor.matmul(fin_ps[:, 10:12], bh, ones2[:, :])
            res = sbt([1, 2], "res")
            nc.vector.tensor_reduce(
                res[:, :],
                fin_ps[:, :].rearrange("p (k b) -> p b k", k=6, b=2),
                mybir.AxisListType.X, ADD)
            nc.sync.dma_start(out=out_e[:, :], in_=res[:, :])

    return nc


def make_in_maps(x, lengths):
    """Per-core input dicts. x: [16,4096,2] f32, lengths: [16] int."""
    x = np.ascontiguousarray(x, dtype=np.float32)
    lengths = np.asarray(lengths).astype(np.int64)
    in_maps = []
    for core in range(NCORES):
        xr4 = np.zeros((128, 32, 4), dtype=np.float32)
        xlast = np.zeros((2, 2), dtype=np.float32)
        for bb in range(NB):
            b = NB * core + bb
            l = min(int(lengths[b]), T_)
            if l < 1:
                # all-zero xr/xlast reproduce the reference's l<=0 output (= bh)
                continue
            full = np.zeros((T_, IN_), dtype=np.float32)
            full[:l] = x[b, l - 1::-1, :]
            xr4[:, :, 2 * bb:2 * bb + 2] = full.reshape(32, 128, 2).transpose(1, 0, 2)
            xlast[:, bb] = x[b, l - 1, :]
        in_maps.append({"xr": xr4.reshape(128, 128), "_xlast": xlast})
    return in_maps


def make_common_inputs(nu_log, theta_log, B_re, B_im, C_re, C_im, D, Wh, bh):
    f = np.float32
    col = lambda v: v.astype(f).reshape(2, 128).T
    halfpair = lambda m: m.astype(f).reshape(2, 128, 2).transpose(1, 0, 2).reshape(128, 4)
    pkrow = np.concatenate([nu_log.astype(f).ravel(),
                            theta_log.astype(f).ravel()]).reshape(1, 512)
    pk = np.zeros((128, 21), dtype=f)
    pk[:, 0:2] = col(nu_log)
    pk[:, 2:4] = col(theta_log)
    pk[:, 4:8] = halfpair(B_re)
    pk[:, 8:12] = halfpair(B_im)
    pk[:, 12:16] = halfpair(D)
    pk[:, 16:18] = col(Wh.reshape(256))
    pk[0, 18] = np.float32(bh.ravel()[0])
    c2 = np.zeros((128, 1024), dtype=f)
    c2[:, 0:512] = C_re.astype(f).reshape(2, 128, 256).transpose(1, 0, 2).reshape(128, 512)
    c2[:, 512:1024] = C_im.astype(f).reshape(2, 128, 256).transpose(1, 0, 2).reshape(128, 512)
    return {"pkrow": np.ascontiguousarray(pkrow),
            "pk": pk, "c2": c2}


def finalize_in_maps(in_maps, common):
    out = []
    for m in in_maps:
        m = dict(m)
        xlast = m.pop("_xlast")
        pk = common["pk"].copy()
        pk[0:2, 19:21] = xlast
        m.update(common)
        m["pk"] = pk
        out.append(m)
    return out


def kernel(x, lengths, nu_log, theta_log, B_re, B_im, C_re, C_im, D, Wh, bh):
    global LAST_RESULT
    x = np.asarray(x)
    common = make_common_inputs(
        np.asarray(nu_log), np.asarray(theta_log), np.asarray(B_re),
        np.asarray(B_im), np.asarray(C_re), np.asarray(C_im), np.asarray(D),
        np.asarray(Wh), np.asarray(bh))
    in_maps = finalize_in_maps(make_in_maps(x, lengths), common)

    global _NC_CACHE
    first_call = _NC_CACHE is None
    if first_call:
        _NC_CACHE = build_graph()
        _NC_CACHE.compile()
    nc = _NC_CACHE
    trace = os.environ.get("BASSLRU_TRACE") == "1"
    LAST_RESULT = run_bass_kernel_spmd(nc, in_maps, core_ids=list(range(NCORES)),
                                       trace=trace)
    if first_call:
        # re-dispatch once after the cold compile+load; the first execution
        # right after NEFF load was observed to be flaky once on this stack
        LAST_RESULT = run_bass_kernel_spmd(nc, in_maps,
                                           core_ids=list(range(NCORES)),
                                           trace=trace)
    outs = [np.asarray(LAST_RESULT.results[c]["out"]).reshape(NB)
            for c in range(NCORES)]
    return np.concatenate(outs).reshape(B_, 1).astype(np.float32)


# revision 60
# speedup vs baseline: 1.0068x; 1.0068x over previous
"""Trainium2 Bass kernel for nn_AddingLRU (LRU scan + masked last-step readout).

Math: the reference's output only depends on the LRU state at t = lengths[b]-1
per batch element:
    out[b] = Re(WC . state_b) + WD . x[b, l_b-1] + bh,
    state_b[n] = sum_{t < l_b} lam[n]^(l_b-1-t) * (Bmat[n,:] . x[b,t,:])
with WC = Wh@C, WD = Wh@D, Bmat = (B_re + i B_im)*gamma.  After a host-side
time-reversal gather (xrev_b[k] = x[b, l_b-1-k]), the state is
    state_b[n] = Bmat[n,:] . sum_k lam[n]^k xrev_b[k,:]
and with k = 128c + j the inner sum factors into a [128,128]x[128,128] matmul
against the power table QT[j,n] = lam[n]^j followed by a weighted reduction
with W[n,c] = lam[n]^(128c).  Both tables are built on-device from
nu_log/theta_log via exp/sin with explicit range reduction to [-pi, pi].

Sharding: data-parallel over batch, 2 batch elements per NeuronCore x 8 cores.
"""
import os
import sys

import numpy as np

for _p in ("/opt/trn_rl_repo", "/root/.axon_site/_ro/trn_rl_repo"):
    if os.path.isdir(_p) and _p not in sys.path:
        sys.path.append(_p)

from concourse import bacc, tile, mybir  # noqa: E402
from concourse.tile import add_dep_helper  # noqa: E402
from concourse.bass_utils import run_bass_kernel_spmd  # noqa: E402

F32 = mybir.dt.float32
TWO_PI = np.float32(6.283185307179586)
INV2PI = np.float32(1.0 / 6.283185307179586)
MAGIC = np.float32(12582912.0)  # 1.5 * 2**23: (x+M)-M == round-to-nearest(x)
SAFE_PI = np.float32(3.141590)  # just under fp32 pi; Sin valid range is [-pi, pi]
HALF_PI = np.float32(1.5707963705062866)

B_, T_, IN_, H_, N_ = 16, 4096, 2, 256, 256
NCORES = 8
NB = B_ // NCORES  # batches per core = 2

LAST_RESULT = None  # BassKernelResults of the most recent run (for test harness)
_NC_CACHE = None  # compiled Bass graph, reused across kernel() calls


def _emit_rsin(nc, sb, out, src, bias, tag, eng=None, eng_head=None):
    """out = sin(src + bias) elementwise, with range reduction to [-pi, pi].

    q = rne((src + bias)/2pi);  r = src - 2pi*q;  out = Sin(clamp(r) + bias).
    bias must be 0.0 or pi/2 (float).  src/out: SBUF APs of equal shape.
    eng_head runs the independent q/k prefix (e.g. on Pool for overlap).
    """
    shape = list(src.shape)
    if eng is None:
        eng = nc.vector
    if eng_head is None:
        eng_head = eng
    q = sb.tile(shape, F32, tag=f"{tag}_q", name=f"{tag}_q")
    if bias == 0.0:
        # q = src*inv2pi + MAGIC  (fused)
        eng_head.tensor_scalar(q[:, :], src, float(INV2PI), float(MAGIC),
                               mybir.AluOpType.mult, mybir.AluOpType.add)
    else:
        boff = np.float32(bias * float(INV2PI))
        eng_head.tensor_scalar(q[:, :], src, float(INV2PI), float(boff),
                               mybir.AluOpType.mult, mybir.AluOpType.add)
        eng_head.tensor_scalar_add(q[:, :], q[:, :], float(MAGIC))
    k = sb.tile(shape, F32, tag=f"{tag}_k", name=f"{tag}_k")
    head_last = eng_head.tensor_scalar_sub(k[:, :], q[:, :], float(MAGIC))
    rr = sb.tile(shape, F32, tag=f"{tag}_r", name=f"{tag}_r")
    # rr = k*(-2pi) + src
    eng.scalar_tensor_tensor(rr[:, :], k[:, :], -float(TWO_PI), src,
                                   mybir.AluOpType.mult, mybir.AluOpType.add)
    # clamp so (rr + bias) stays inside [-pi, pi] for the Sin table
    hi = float(SAFE_PI) - float(bias)
    lo = -float(SAFE_PI) - float(bias)
    last = eng.tensor_scalar(rr[:, :], rr[:, :], hi, lo,
                             mybir.AluOpType.min, mybir.AluOpType.max)
    if bias != 0.0:
        last = eng.tensor_scalar_add(rr[:, :], rr[:, :], float(bias))
    return (nc.scalar.activation(out, rr[:, :],
                                 mybir.ActivationFunctionType.Sin),
            last, head_last, rr)


def build_graph():
    nc = bacc.Bacc(None, target_bir_lowering=False, debug=False)

    # ---- DRAM parameters (host packs small tensors to minimize DMA count)
    # pkrow [1, 512]: nu_log row (0:256) | theta_log row (256:512)
    # pk [128, 21]: nulog_col(0:2) thetalog_col(2:4) bre(4:8) bim(8:12)
    #              d(12:16) wh(16:18) bh(p0, 18) xlast(p0:2, 19:21)
    # c2 [128, 1024]: cre (0:512) | cim (512:1024)
    pkrow_e = nc.declare_dram_parameter("pkrow", [1, 512], F32, isOutput=False)
    pk_e = nc.declare_dram_parameter("pk", [128, 21], F32, isOutput=False)
    xr_e = nc.declare_dram_parameter("xr", [128, 128], F32, isOutput=False)
    c2_e = nc.declare_dram_parameter("c2", [128, 1024], F32, isOutput=False)
    out_e = nc.declare_dram_parameter("out", [1, 2], F32, isOutput=True)

    MULT = mybir.AluOpType.mult
    ADD = mybir.AluOpType.add
    Exp = mybir.ActivationFunctionType.Exp
    Ln = mybir.ActivationFunctionType.Ln

    with tile.TileContext(nc) as tc:
        with tc.tile_pool(name="sb", bufs=1) as sb, \
             tc.tile_pool(name="ps", bufs=1, space="PSUM") as ps:
            def sbt(shape, tag):
                return sb.tile(shape, F32, tag=tag, name=tag)

            # ---- DMA inputs (critical-path first) ----
            # broadcast nu_log/theta_log rows across partitions straight from
            # DRAM (stride-0 free-dim read) -- no ACT dependency, issues at t=0
            bc_thlog = sbt([128, 256], "bc_thlog")
            nc.sync.dma_start(out=bc_thlog[:, :],
                              in_=pkrow_e[0:1, 256:512].unsqueeze(1)
                              .broadcast_to((1, 128, 256)))
            bc_nulog = sbt([128, 256], "bc_nulog")
            nc.sync.dma_start(out=bc_nulog[:, :],
                              in_=pkrow_e[0:1, 0:256].unsqueeze(1)
                              .broadcast_to((1, 128, 256)))
            pk = sbt([128, 21], "pk")
            nc.sync.dma_start(out=pk[:, :], in_=pk_e[:, :])
            xr = sbt([128, 128], "xr")
            c2 = sbt([128, 1024], "c2")
            nulc = pk[:, 0:2]
            thlc = pk[:, 2:4]
            bre_ap = pk[:, 4:8]
            bim_ap = pk[:, 8:12]
            dsb = pk[:, 12:16]
            wh = pk[:, 16:18]
            bh = pk[0:1, 18:19]
            xlast = pk[0:2, 19:21]
            cre = c2[:, 0:512]
            cim = c2[:, 512:1024]

            # ---- constants ----
            t_col = sbt([128, 1], "t_col")
            nc.gpsimd.iota(t_col[:, :], [[1, 1]], channel_multiplier=1,
                           allow_small_or_imprecise_dtypes=True)
            iota_w = sbt([128, 32], "iota_w")  # value = 128*c, same per partition
            nc.gpsimd.iota(iota_w[:, :], [[128, 32]], channel_multiplier=0,
                           allow_small_or_imprecise_dtypes=True)
            ones_col = sbt([128, 1], "ones_col")
            nc.gpsimd.memset(ones_col[:, :], 1.0)

            # ---- exp-family ACT batch ----
            th_bc = sbt([128, 256], "th_bc")  # theta broadcast [t, n]
            e_i2 = nc.scalar.activation(th_bc[:, :], bc_thlog[:, :], Exp)
            nu_bc = sbt([128, 256], "nu_bc")
            e_i1 = nc.scalar.activation(nu_bc[:, :], bc_nulog[:, :], Exp)
            # nu/theta column halves in one exp (adjacent pk columns)
            ath_col = sbt([128, 4], "ath_col")
            e_i3 = nc.scalar.activation(ath_col[:, :], pk[:, 0:4], Exp)
            a_col = ath_col[:, 0:2]
            th_col = ath_col[:, 2:4]
            e_i4 = e_i3
            # th_bc/nu_bc gate the DVE phase chain; keep them ahead of the
            # pk-gated column exp in the in-order ACT stream
            for early in (e_i2, e_i1):
                add_dep_helper(e_i3.ins, early.ins, sync=False,
                               reason="broadcast exps first on ACT")
            negnu = sbt([128, 2], "negnu")
            negnu_i = nc.gpsimd.tensor_scalar_mul(negnu[:, :], a_col, -1.0)
            e2 = sbt([128, 2], "e2")  # exp(-2 nu) = |lam|^2
            e_i5 = nc.scalar.activation(e2[:, :], a_col, Exp, scale=-2.0)
            # gamma = sqrt(u), u = 1-e2, via bit-trick rsqrt + 2 Newton steps
            # (avoids the Ln/Sqrt ACT tables; only exp+sin sets get loaded)
            SHR = mybir.AluOpType.arith_shift_right
            u_g = sbt([128, 2], "u_g")
            u_g_i = nc.gpsimd.tensor_scalar(u_g[:, :], e2[:, :], -1.0, 1.0, MULT, ADD)
            uh_g = sbt([128, 2], "uh_g")
            nc.gpsimd.tensor_scalar_mul(uh_g[:, :], u_g[:, :], 0.5)
            yi_g = sb.tile([128, 2], mybir.dt.int32, tag="yi_g", name="yi_g")
            yi_i1 = nc.vector.tensor_scalar(yi_g[:, :],
                                            u_g[:, :].bitcast(mybir.dt.int32),
                                            1, None, SHR)
            # y0 bits = magic - (bits >> 1)
            yi_i2 = nc.vector.tensor_scalar(yi_g[:, :], yi_g[:, :], -1,
                                            0x5f3759df, MULT, ADD)
            y_g = sbt([128, 2], "y_g")
            nc.gpsimd.tensor_copy(y_g[:, :], yi_g[:, :].bitcast(F32))
            t_g = sbt([128, 2], "t_g")
            for _ in range(2):
                nc.gpsimd.tensor_mul(t_g[:, :], y_g[:, :], y_g[:, :])
                nc.gpsimd.tensor_mul(t_g[:, :], t_g[:, :], uh_g[:, :])
                nc.gpsimd.tensor_scalar(t_g[:, :], t_g[:, :], -1.0, 1.5, MULT, ADD)
                nc.gpsimd.tensor_mul(y_g[:, :], y_g[:, :], t_g[:, :])
            gamma = sbt([128, 2], "gamma")
            nc.gpsimd.tensor_mul(gamma[:, :], u_g[:, :], y_g[:, :])

            # bulk DMAs (needed much later) -- on the Pool DMA queue so their
            # completion semaphore doesn't gate the critical ACT ops
            nc.sync.dma_start(out=xr[:, :], in_=xr_e[:, :])
            nc.sync.dma_start(out=c2[:, :], in_=c2_e[:, :])
            bc_th = th_bc[:, :]
            bc_nu = nu_bc[:, :]

            # ---- phases & magnitudes, batched in one [128, 320] tile ----
            # cols 0:256   -> QT:  phase j*theta[n],  mag exp(-j*nu[n])
            # cols 256+32h -> W_h: phase 128c*theta_h, mag exp(-128c*nu_h)
            Cp = mybir.ActivationFunctionType.Copy
            P_all = sbt([128, 320], "P_all")
            nc.vector.tensor_scalar(P_all[:, 0:256], bc_th, t_col[:, 0:1],
                                    None, MULT)
            for h in (0, 1):
                nc.vector.tensor_scalar(P_all[:, 256 + 32 * h:288 + 32 * h],
                                        iota_w[:, :], th_col[:, h:h + 1],
                                        None, MULT)
            negt = sbt([128, 1], "negt")
            nc.gpsimd.tensor_scalar_mul(negt[:, :], t_col[:, :], -1.0)
            E_all = sbt([128, 320], "E_all")
            e_i8 = nc.scalar.activation(E_all[:, 0:256], bc_nu, Exp,
                                        scale=negt[:, 0:1])
            exp_insts = [e_i1, e_i2, e_i3, e_i5, e_i8]
            for h in (0, 1):
                exp_insts.append(
                    nc.scalar.activation(E_all[:, 256 + 32 * h:288 + 32 * h],
                                         iota_w[:, :], Exp,
                                         scale=negnu[:, h:h + 1]))
            # PRD packs both tables: [0:320] = re = E*cos, [320:640] = im = E*sin
            # QTre = PRD[:, 0:256], QTim = PRD[:, 320:576]
            # Wre_h = PRD[:, 256+32h], Wim_h = PRD[:, 576+32h]  (im = re + 320)
            PRD = sbt([128, 640], "PRD")
            sinA = sbt([128, 320], "sinA")
            sin_i1, _, _, rr_sin = _emit_rsin(nc, sb, sinA[:, :], P_all[:, :],
                                             0.0, "sa")
            # PRD-im lands between the two rsin chains in the DVE stream so it
            # overlaps cosA's ACT call and unblocks the imaginary matmuls early
            nc.vector.tensor_mul(PRD[:, 320:640], E_all[:, :], sinA[:, :])
            # cos(P) = cos(|r_sin|) = sin(pi/2 - |r_sin|): reuse the sin
            # chain's reduced argument (2 DVE ops instead of a second chain)
            cosA = sbt([128, 320], "cosA")
            cabs = sb.tile([128, 320], mybir.dt.int32, tag="cabs", name="cabs")
            nc.vector.tensor_scalar(cabs[:, :],
                                    rr_sin[:, :].bitcast(mybir.dt.int32),
                                    0x7fffffff, None,
                                    mybir.AluOpType.bitwise_and)
            carg = sbt([128, 320], "carg")
            cos_last = nc.vector.tensor_scalar(carg[:, :],
                                               cabs[:, :].bitcast(F32),
                                               -1.0, float(HALF_PI), MULT, ADD)
            sin_i2 = nc.scalar.activation(cosA[:, :], carg[:, :],
                                          mybir.ActivationFunctionType.Sin)
            # -Wim for the T1 pt1 products (needs only the im half of PRD)
            PRDn = sbt([128, 64], "PRDn")
            nc.vector.tensor_scalar_mul(PRDn[:, :], PRD[:, 576:640], -1.0)
            nc.vector.tensor_mul(PRD[:, 0:320], E_all[:, :], cosA[:, :])
            for ei in exp_insts:
                add_dep_helper(sin_i1.ins, ei.ins, sync=False,
                               reason="batch exp table before sin table")
                add_dep_helper(sin_i2.ins, ei.ins, sync=False,
                               reason="batch exp table before sin table")
            # keep the off-critical-path int rsqrt ops behind the rsin chains
            add_dep_helper(yi_i1.ins, cos_last.ins, sync=False,
                           reason="defer rsqrt int ops on DVE")
            add_dep_helper(yi_i2.ins, cos_last.ins, sync=False,
                           reason="defer rsqrt int ops on DVE")

            # ---- main matmuls: hps[h][pt][n, (c, bi)] = QT^T @ xr ----
            # four PSUM tiles so each consumer waits only on its own matmul
            hps = [[None, None], [None, None]]
            hsb = [[None, None], [None, None]]  # SBUF bounce for Pool (no PSUM)
            hsb_last = None
            for pt in (1, 0):  # imaginary first: sinA finishes before cosA
                for h in (0, 1):
                    t = ps.tile([128, 128], F32, tag=f"hps{h}{pt}",
                                name=f"hps{h}{pt}")
                    hps[h][pt] = t
                    q = 320 * pt + 128 * h
                    nc.tensor.matmul(t[:, :], PRD[:, q:q + 128], xr[:, :])
                    hsb[h][pt] = sbt([128, 128], f"hsb{h}{pt}")
                    hsb_last = nc.scalar.activation(hsb[h][pt][:, :], t[:, :], Cp)
                    add_dep_helper(hsb_last.ins, sin_i2.ins, sync=False,
                                   reason="bounce copies after sins on ACT")


            # ---- combine: g = sum_c W * h (complex) ----
            # gre = sum_c (Wre*hre + (-Wim)*him): with -Wim folded into the
            # product weights, the (pt, c) reduce yields gre directly.
            # per-(h, pt) products into per-h tiles T1t[h] (f = pt*128+c*4+bi)
            T1t = [sbt([128, 256], "T1t0"), sbt([128, 256], "T1t1")]
            T2t = [sbt([128, 256], "T2t0"), sbt([128, 256], "T2t1")]
            wre_ap = lambda h: PRD[:, 256 + 32 * h:288 + 32 * h] \
                .rearrange("p c -> p c").unsqueeze(2).broadcast_to((128, 32, 4))
            wim_ap = lambda h: PRD[:, 576 + 32 * h:608 + 32 * h] \
                .unsqueeze(2).broadcast_to((128, 32, 4))
            wimn_ap = lambda h: PRDn[:, 32 * h:32 * h + 32] \
                .unsqueeze(2).broadcast_to((128, 32, 4))
            def t1v(h):
                return T1t[h][:, :].rearrange("p (pt c b) -> p pt c b",
                                              pt=2, c=32, b=4)

            def t2v(h):
                return T2t[h][:, :].rearrange("p (pt c b) -> p pt c b",
                                              pt=2, c=32, b=4)

            def hv(h, pt):
                return hps[h][pt][:, :].rearrange("p (c b) -> p c b", c=32, b=4)

            # T1 (DVE): pt0 = Wre*hre, pt1 = (-Wim)*him; im products first
            for h in (0, 1):
                nc.vector.tensor_tensor(t1v(h)[:, 1, :, :], wimn_ap(h),
                                        hv(h, 1), MULT)
            for h in (0, 1):
                nc.vector.tensor_tensor(t1v(h)[:, 0, :, :], wre_ap(h),
                                        hv(h, 0), MULT)
            # T2 (Pool): pt0 = Wre*him, pt1 = Wim*hre; him-based first
            def hvs(h, pt):
                return hsb[h][pt][:, :].rearrange("p (c b) -> p c b", c=32, b=4)

            t2_last = None
            for pt in (0, 1):
                for h in (0, 1):
                    t2_last = nc.gpsimd.tensor_tensor(
                        t2v(h)[:, pt, :, :],
                        wre_ap(h) if pt == 0 else wim_ap(h),
                        hvs(h, 1 - pt), MULT)
            # g2 = [gre | gim], each (h, bb, i); reduce over (pt, c)
            g2 = sbt([128, 16], "g2")
            for h in (0, 1):
                nc.vector.tensor_reduce(
                    g2[:, 4 * h:4 * h + 4],
                    T1t[h][:, :].rearrange("p (pt c b) -> p b pt c",
                                           pt=2, c=32, b=4),
                    mybir.AxisListType.XY, ADD)
            for h in (0, 1):
                nc.vector.tensor_reduce(
                    g2[:, 8 + 4 * h:12 + 4 * h],
                    T2t[h][:, :].rearrange("p (pt c b) -> p b pt c",
                                           pt=2, c=32, b=4),
                    mybir.AxisListType.XY, ADD)

            # ---- Bmat fold: state[n, (h, bb)] (prep is off the critical path)
            breg = sbt([128, 4], "breg")
            bimg = sbt([128, 4], "bimg")
            for h in (0, 1):
                bi1 = nc.scalar.activation(breg[:, 2 * h:2 * h + 2],
                                           bre_ap[:, 2 * h:2 * h + 2], Cp,
                                           scale=gamma[:, h:h + 1])
                bi2 = nc.scalar.activation(bimg[:, 2 * h:2 * h + 2],
                                           bim_ap[:, 2 * h:2 * h + 2], Cp,
                                           scale=gamma[:, h:h + 1])
                for late in (bi1, bi2):
                    add_dep_helper(late.ins, sin_i1.ins, sync=False,
                                   reason="keep crit-path ACTs ahead")
                    add_dep_helper(late.ins, hsb_last.ins, sync=False,
                                   reason="bounce copies first on ACT")
            bimgn = sbt([128, 4], "bimgn")
            nc.gpsimd.tensor_scalar_mul(bimgn[:, :], bimg[:, :], -1.0)
            # critical: P16a = g2 * [bregf | -bimgf], P16b = g2 * [bimgf | bregf]
            # bb-broadcast of breg/bimg(n) via stride-0 views (no materialization)
            bwv = lambda t: t[:, :].rearrange("p (h i) -> p h i", h=2, i=2)                 .unsqueeze(2).broadcast_to((128, 2, 2, 2))                 .rearrange("p h bb i -> p h bb i")
            g2v = g2[:, :].rearrange("p (pt h bb i) -> p pt h bb i",
                                     pt=2, h=2, bb=2, i=2)
            P16a = sbt([128, 16], "P16a")
            p16av = P16a[:, :].rearrange("p (pt h bb i) -> p pt h bb i",
                                         pt=2, h=2, bb=2, i=2)
            nc.vector.tensor_tensor(p16av[:, 0], g2v[:, 0], bwv(breg), MULT)
            nc.vector.tensor_tensor(p16av[:, 1], g2v[:, 1], bwv(bimgn), MULT)
            P16b = sbt([128, 16], "P16b")
            p16bv = P16b[:, :].rearrange("p (pt h bb i) -> p pt h bb i",
                                         pt=2, h=2, bb=2, i=2)
            nc.vector.tensor_tensor(p16bv[:, 0], g2v[:, 1], bwv(breg), MULT)
            nc.vector.tensor_tensor(p16bv[:, 1], g2v[:, 0], bwv(bimg), MULT)
            st2 = sbt([128, 8], "st2")
            nc.vector.tensor_reduce(
                st2[:, 0:4],
                P16a[:, :].rearrange("p (pt hb i) -> p hb pt i",
                                     pt=2, hb=4, i=2),
                mybir.AxisListType.XY, ADD)
            nc.vector.tensor_reduce(
                st2[:, 4:8],
                P16b[:, :].rearrange("p (pt hb i) -> p hb pt i",
                                     pt=2, hb=4, i=2),
                mybir.AxisListType.XY, ADD)

            # ---- WC = Wh @ C as columns [n-half], WD = Wh @ D ----
            wc_ps = ps.tile([128, 4], F32, tag="wc_ps", name="wc_ps")  # cols: pt*2 + h
            for pt, csb in ((0, cre), (1, cim)):
                for h in (0, 1):
                    col = 2 * pt + h
                    for hc in (0, 1):
                        o = 256 * hc + 128 * h
                        nc.tensor.matmul(wc_ps[:, col:col + 1],
                                         csb[:, o:o + 128], wh[:, hc:hc + 1],
                                         start=(hc == 0), stop=(hc == 1))
            # w_fin = [wcre | -wcim] bb-duplicated; f = pt*4 + h*2 + bb
            w_fin = sbt([128, 8], "w_fin")
            wf_v = w_fin[:, :].rearrange("p (pt h bb) -> p pt bb h",
                                         pt=2, h=2, bb=2)
            for bb in (0, 1):
                wfi1 = nc.scalar.activation(wf_v[:, 0, bb, :], wc_ps[:, 0:2], Cp)
                wfi2 = nc.scalar.activation(wf_v[:, 1, bb, :], wc_ps[:, 2:4], Cp,
                                            scale=-1.0)
                add_dep_helper(wfi1.ins, hsb_last.ins, sync=False,
                               reason="keep crit-path ACTs ahead")
                add_dep_helper(wfi2.ins, hsb_last.ins, sync=False,
                               reason="keep crit-path ACTs ahead")

            wd_ps = ps.tile([2, 1], F32, tag="wd_ps", name="wd_ps")
            for hc in (0, 1):
                nc.tensor.matmul(wd_ps[:, :], dsb[:, 2 * hc:2 * hc + 2],
                                 wh[:, hc:hc + 1],
                                 start=(hc == 0), stop=(hc == 1))
            wd_sb = sbt([2, 1], "wd_sb")
            wdi = nc.scalar.activation(wd_sb[:, :], wd_ps[:, :], Cp)
            add_dep_helper(wdi.ins, hsb_last.ins, sync=False,
                           reason="keep crit-path ACTs ahead")

            # ---- final: P8 = st2*w_fin; one PE reduce + one DVE reduce ----
            P8 = sbt([128, 8], "P8")
            nc.vector.tensor_mul(P8[:, :], st2[:, :], w_fin[:, :])
            ones2 = sbt([1, 2], "ones2")
            nc.gpsimd.memset(ones2[:, :], 1.0)
            fin_ps = ps.tile([1, 12], F32, tag="fin_ps", name="fin_ps")
            nc.tensor.matmul(fin_ps[:, 0:8], ones_col[:, :], P8[:, :])
            nc.tensor.matmul(fin_ps[:, 8:10], wd_sb[:, :], xlast)
            nc.tensor.matmul(fin_ps[:, 10:12], bh, ones2[:, :])
            res = sbt([1, 2], "res")
            nc.vector.tensor_reduce(
                res[:, :],
                fin_ps[:, :].rearrange("p (k b) -> p b k", k=6, b=2),
                mybir.AxisListType.X, ADD)
            nc.sync.dma_start(out=out_e[:, :], in_=res[:, :])

    return nc


def make_in_maps(x, lengths):
    """Per-core input dicts. x: [16,4096,2] f32, lengths: [16] int."""
    x = np.ascontiguousarray(x, dtype=np.float32)
    lengths = np.asarray(lengths).astype(np.int64)
    in_maps = []
    for core in range(NCORES):
        xr4 = np.zeros((128, 32, 4), dtype=np.float32)
        xlast = np.zeros((2, 2), dtype=np.float32)
        for bb in range(NB):
            b = NB * core + bb
            l = min(int(lengths[b]), T_)
            if l < 1:
                # all-zero xr/xlast reproduce the reference's l<=0 output (= bh)
                continue
            full = np.zeros((T_, IN_), dtype=np.float32)
            full[:l] = x[b, l - 1::-1, :]
            xr4[:, :, 2 * bb:2 * bb + 2] = full.reshape(32, 128, 2).transpose(1, 0, 2)
            xlast[:, bb] = x[b, l - 1, :]
        in_maps.append({"xr": xr4.reshape(128, 128), "_xlast": xlast})
    return in_maps


def make_common_inputs(nu_log, theta_log, B_re, B_im, C_re, C_im, D, Wh, bh):
    f = np.float32
    col = lambda v: v.astype(f).reshape(2, 128).T
    halfpair = lambda m: m.astype(f).reshape(2, 128, 2).transpose(1, 0, 2).reshape(128, 4)
    pkrow = np.concatenate([nu_log.astype(f).ravel(),
                            theta_log.astype(f).ravel()]).reshape(1, 512)
    pk = np.zeros((128, 21), dtype=f)
    pk[:, 0:2] = col(nu_log)
    pk[:, 2:4] = col(theta_log)
    pk[:, 4:8] = halfpair(B_re)
    pk[:, 8:12] = halfpair(B_im)
    pk[:, 12:16] = halfpair(D)
    pk[:, 16:18] = col(Wh.reshape(256))
    pk[0, 18] = np.float32(bh.ravel()[0])
    c2 = np.zeros((128, 1024), dtype=f)
    c2[:, 0:512] = C_re.astype(f).reshape(2, 128, 256).transpose(1, 0, 2).reshape(128, 512)
    c2[:, 512:1024] = C_im.astype(f).reshape(2, 128, 256).transpose(1, 0, 2).reshape(128, 512)
    return {"pkrow": np.ascontiguousarray(pkrow),
            "pk": pk, "c2": c2}


def finalize_in_maps(in_maps, common):
    out = []
    for m in in_maps:
        m = dict(m)
        xlast = m.pop("_xlast")
        pk = common["pk"].copy()
        pk[0:2, 19:21] = xlast
        m.update(common)
        m["pk"] = pk
        out.append(m)
    return out


def kernel(x, lengths, nu_log, theta_log, B_re, B_im, C_re, C_im, D, Wh, bh):
    global LAST_RESULT
    x = np.asarray(x)
    common = make_common_inputs(
        np.asarray(nu_log), np.asarray(theta_log), np.asarray(B_re),
        np.asarray(B_im), np.asarray(C_re), np.asarray(C_im), np.asarray(D),
        np.asarray(Wh), np.asarray(bh))
    in_maps = finalize_in_maps(make_in_maps(x, lengths), common)

    global _NC_CACHE
    first_call = _NC_CACHE is None
    if first_call:
        _NC_CACHE = build_graph()
        _NC_CACHE.compile()
    nc = _NC_CACHE
    trace = os.environ.get("BASSLRU_TRACE") == "1"
    LAST_RESULT = run_bass_kernel_spmd(nc, in_maps, core_ids=list(range(NCORES)),
                                       trace=trace)
    if first_call:
        # re-dispatch once after the cold compile+load; the first execution
        # right after NEFF load was observed to be flaky once on this stack
        LAST_RESULT = run_bass_kernel_spmd(nc, in_maps,
                                           core_ids=list(range(NCORES)),
                                           trace=trace)
    outs = [np.asarray(LAST_RESULT.results[c]["out"]).reshape(NB)
            for c in range(NCORES)]
    return np.concatenate(outs).reshape(B_, 1).astype(np.float32)
